# revision 13
# baseline (speedup 1.0000x reference)
"""Equivariant LayerNorm (128x0e + 64x1o + 32x2e) Trainium2 Bass kernel.

Sharding: pure data parallel over 8 NeuronCores, 32768 rows each.

Key design (v2 — engineered against measured per-engine rates):
  * Host permutes the v1/v2 feature blocks to k-major ("interleaved") order so
    every per-segment broadcast on the device has 64/32-wide consecutive inner
    runs -> fp16 tensor_tensor ops hit the DVE 2x packed mode.
  * Output is stored as fp16 (rel err ~8e-4, gate 2e-2) halving store traffic.
  * v1/v2 blocks: two-pass variance. Pass 1 sums in f32 (mean must be exact:
    tiny-variance segments amplify mean error by 1/sqrt(var+eps)).  The
    centered tensor xc is cached as fp16 (no cancellation once centered), so
    the square, the sum-of-squares trees and the normalize multiply all run
    at 2x on fp16.
  * scal block (d=128): single-pass f32 variance (E[x^2]-m^2 is safe at d=128
    since row variance is never tiny) + per-row-fused ACT normalize
    t = Identity(x*inv + (-m*inv)) with per-partition scale/bias.
  * GPSIMD takes the two big mixed-dtype centering adds (f32+f32->fp16, 1x on
    DVE anyway); ScalarE takes squares + rsqrt + the fused scal normalize.
"""

import sys

import numpy as np

try:
    import concourse  # noqa: F401
except ImportError:  # pragma: no cover
    sys.path.insert(0, "/opt/trn_rl_repo")

from contextlib import ExitStack

import concourse.bacc as bacc
import concourse.bass as bass
import concourse.mybir as mybir
import concourse.tile as tile
from concourse.bass_utils import run_bass_kernel_spmd

F32 = mybir.dt.float32
F16 = mybir.dt.float16
AF = mybir.ActivationFunctionType
AXX = mybir.AxisListType.X
ALU = mybir.AluOpType

N = 262144
DIM = 480
S = 128
G1, D1 = 64, 3
G2, D2 = 32, 5
GS = 1 + G1 + G2  # 97 segments per row (seg 0 = the 128 scalar cols)
EPS = 1e-5

N_CORES = 8
PRI_LOAD = 100   # schedule loads ~2 tiles early
PRI_STAGE = 50   # sums/center/squares ~1 tile early
ROWS = N // N_CORES  # 32768
B = 8  # row-blocks per SBUF tile
TILE_ROWS = 128 * B

# engine assignment knobs (tuned against HW)
ENG_C1 = "gpsimd"  # center v1
ENG_C2 = "gpsimd"  # center v2
ENG_E2 = "vector"  # SS_v1 tree
ENG_E3 = "vector"  # SS_v2 tree
ENG_XSQS = "scalar"  # scal x^2 (f32)


def _perm():
    """Device column permutation: v-blocks to k-major (interleaved) order.
    perm[dev_col] = orig_col."""
    p = np.arange(DIM)
    for k in range(D1):
        for g in range(G1):
            p[S + k * G1 + g] = S + g * D1 + k
    off = S + G1 * D1
    for k in range(D2):
        for g in range(G2):
            p[off + k * G2 + g] = off + g * D2 + k
    return p


PERM = _perm()


def _rsqrt(nc, out_ap, in_ap, bias_ap):
    """out = 1/sqrt(in + bias) on ScalarE. The bass wrapper rejects Rsqrt on
    accuracy grounds; measured on this HW it is ~4e-5 max rel err, far below
    the tolerance here."""
    eng = nc.scalar
    return eng.add_instruction(
        mybir.InstActivation(
            name=nc.get_next_instruction_name(),
            func=AF.Rsqrt,
            ins=[
                eng.lower_ap(in_ap),
                eng.lower_ap(bias_ap),
                mybir.ImmediateValue(dtype=F32, value=1.0),
                mybir.ImmediateValue(dtype=F32, value=0.0),
            ],
            outs=[eng.lower_ap(out_ap)],
        )
    )


def build_nc(rows=ROWS, b_blocks=B):
    nc = bacc.Bacc("TRN2", target_bir_lowering=False, debug=False)
    Bb = b_blocks
    trows = 128 * Bb
    assert rows % trows == 0
    ntiles = rows // trows
    V1W = G1 * D1  # 192
    V2W = G2 * D2  # 160
    VW = V1W + V2W  # 352

    x_d = nc.dram_tensor("x", [rows, DIM], F32, kind="ExternalInput").ap()
    wb_d = nc.dram_tensor("wb", [128, b_blocks * S], F16, kind="ExternalInput").ap()
    bb_d = nc.dram_tensor("bb", [128, b_blocks * S], F16, kind="ExternalInput").ap()
    eps_d = nc.dram_tensor("epsv", [128, 1], F32, kind="ExternalInput").ap()
    out_d = nc.dram_tensor("out", [rows, DIM], F16, kind="ExternalOutput").ap()

    # p-major row blocking: row = n*(128*B) + p*B + b -> each partition's tile
    # slice is one contiguous run in DRAM (fat DMA descriptors)
    xv = x_d.rearrange("(n p b) f -> n p (b f)", p=128, b=Bb)
    ov = out_d.rearrange("(n p b) f -> n p (b f)", p=128, b=Bb)

    def eng(name):
        return getattr(nc, name)

    with tile.TileContext(nc) as tc, ExitStack() as ctx:
        const = ctx.enter_context(tc.tile_pool(name="const", bufs=1))
        px = ctx.enter_context(tc.tile_pool(name="px", bufs=4))
        pxc = ctx.enter_context(tc.tile_pool(name="pxc", bufs=4))
        pxsq = ctx.enter_context(tc.tile_pool(name="pxsq", bufs=3))
        po = ctx.enter_context(tc.tile_pool(name="po", bufs=3))
        pst = ctx.enter_context(tc.tile_pool(name="pst", bufs=3))

        wb_t = const.tile([128, Bb * S], F16, name="wbt", tag="wb")
        nc.sync.dma_start(wb_t[:], wb_d)
        bb_t = const.tile([128, Bb * S], F16, name="bbt", tag="bb")
        nc.sync.dma_start(bb_t[:], bb_d)
        eps_t = const.tile([128, 1], F32, name="epst", tag="epsv")
        nc.sync.dma_start(eps_t[:], eps_d)

        wb_b = wb_t[:].rearrange("p (b f) -> p b f", b=Bb)
        bb_b = bb_t[:].rearrange("p (b f) -> p b f", b=Bb)

        for i in range(ntiles):
            xt = px.tile([128, Bb * DIM], F32, name="xt", tag="x")
            with tc.high_priority(offset=PRI_LOAD):
                nc.sync.dma_start(xt[:], xv[i])
            x3 = xt[:].rearrange("p (b f) -> p b f", b=Bb)
            x_s = x3[:, :, 0:S]
            x_v1 = x3[:, :, S : S + V1W].rearrange("p b (k g) -> p b k g", k=D1)
            x_v2 = x3[:, :, S + V1W : DIM].rearrange("p b (k g) -> p b k g", k=D2)

            # ---- pass 1: segment sums (f32) ----
            hp = tc.high_priority(offset=PRI_STAGE)
            hp.__enter__()
            nSs = pst.tile([128, Bb], F32, name="nSs", tag="nSs")  # -sum(scal)
            nc.vector.tensor_reduce(
                nSs[:], x_s, axis=AXX, op=ALU.add, negate=True
            )
            nm_s = pst.tile([128, Bb], F32, name="nm_s", tag="nm_s")
            nc.vector.tensor_scalar_mul(nm_s[:], nSs[:], 1.0 / S)

            tr1 = pst.tile([128, Bb * G1], F32, name="tr1", tag="tr1")
            t14 = tr1[:].rearrange("p (b h g) -> p b h g", b=Bb, h=2)
            xv14 = x_v1.rearrange("p b k (h g) -> p b k h g", h=2)
            nc.vector.tensor_add(t14, xv14[:, :, 0, :, :], xv14[:, :, 1, :, :])
            Sv1 = pst.tile([128, Bb * G1], F32, name="Sv1", tag="Sv1")
            S14 = Sv1[:].rearrange("p (b h g) -> p b h g", b=Bb, h=2)
            nc.vector.tensor_add(S14, t14, xv14[:, :, 2, :, :])
            nm_v1 = pst.tile([128, Bb * G1], F32, name="nm_v1", tag="nm_v1")
            nc.vector.tensor_scalar_mul(nm_v1[:], Sv1[:], -1.0 / D1)
            nm1_b = (
                nm_v1[:]
                .rearrange("p (b o g) -> p b o g", b=Bb, o=1)
                .broadcast_to([128, Bb, D1, G1])
            )

            ra = pst.tile([128, Bb * 2 * G2], F32, name="ra", tag="ra")
            rc = pst.tile([128, Bb * G2], F32, name="rc", tag="rc")
            rc3 = rc[:].rearrange("p (b g) -> p b g", b=Bb)
            Sv2 = pst.tile([128, Bb * G2], F32, name="Sv2", tag="Sv2")
            S23 = Sv2[:].rearrange("p (b g) -> p b g", b=Bb)
            rab = ra[:].rearrange("p (b k g) -> p b k g", b=Bb, k=2)
            nc.vector.tensor_add(
                rab, x_v2[:, :, 0:4:2, :], x_v2[:, :, 1:4:2, :]
            )
            nc.vector.tensor_add(rc3, rab[:, :, 0, :], rab[:, :, 1, :])
            nc.vector.tensor_add(S23, rc3, x_v2[:, :, 4, :])
            nm_v2 = pst.tile([128, Bb * G2], F32, name="nm_v2", tag="nm_v2")
            nc.vector.tensor_scalar_mul(nm_v2[:], Sv2[:], -1.0 / D2)
            nm2_b = (
                nm_v2[:]
                .rearrange("p (b o g) -> p b o g", b=Bb, o=1)
                .broadcast_to([128, Bb, D2, G2])
            )

            # ---- center v-blocks -> fp16 xc (compact [p, b, 352]) ----
            xc = pxc.tile([128, Bb * VW], F16, name="xc", tag="xc")
            c3 = xc[:].rearrange("p (b f) -> p b f", b=Bb)
            xc_v1 = c3[:, :, 0:V1W].rearrange("p b (k g) -> p b k g", k=D1)
            xc_v2 = c3[:, :, V1W:VW].rearrange("p b (k g) -> p b k g", k=D2)
            eng(ENG_C1).tensor_add(xc_v1, x_v1, nm1_b)
            eng(ENG_C2).tensor_add(xc_v2, x_v2, nm2_b)

            # ---- squares ----
            xsqs = pxsq.tile([128, Bb * S], F32, name="xsqs", tag="xsqs")
            sq3 = xsqs[:].rearrange("p (b f) -> p b f", b=Bb)
            # scale 1/sqrt(S): the reduce then yields SS/S = E[x^2] directly
            nc.scalar.activation(sq3, x_s, AF.Square, scale=float(1.0 / np.sqrt(S)))
            xsq = pxsq.tile([128, Bb * VW], F16, name="xsq", tag="xsq")
            q3 = xsq[:].rearrange("p (b f) -> p b f", b=Bb)
            nc.scalar.activation(
                q3[:, :, 0:V1W], c3[:, :, 0:V1W], AF.Square,
                scale=float(1.0 / np.sqrt(D1)),
            )
            nc.scalar.activation(
                q3[:, :, V1W:VW], c3[:, :, V1W:VW], AF.Square,
                scale=float(1.0 / np.sqrt(D2)),
            )
            xsq_v1 = q3[:, :, 0:V1W].rearrange("p b (k g) -> p b k g", k=D1)
            xsq_v2 = q3[:, :, V1W:VW].rearrange("p b (k g) -> p b k g", k=D2)
            hp.__exit__(None, None, None)

            # ---- pass 2: sums of squares ----
            SSs = pst.tile([128, Bb], F32, name="SSs", tag="SSs")
            nc.vector.tensor_reduce(SSs[:], sq3, axis=AXX, op=ALU.add)

            GVv = G1 + G2
            var96 = pst.tile([128, Bb * GVv], F16, name="var96", tag="var96")
            v963 = var96[:].rearrange("p (b g) -> p b g", b=Bb)
            tq1 = pst.tile([128, Bb * G1], F16, name="tq1", tag="tq1")
            tq14 = tq1[:].rearrange("p (b h g) -> p b h g", b=Bb, h=2)
            xq14 = xsq_v1.rearrange("p b k (h g) -> p b k h g", h=2)
            eng(ENG_E2).tensor_add(tq14, xq14[:, :, 0, :, :], xq14[:, :, 1, :, :])
            v96h = v963[:, :, 0:G1].rearrange("p b (h g) -> p b h g", h=2)
            eng(ENG_E2).tensor_add(v96h, tq14, xq14[:, :, 2, :, :])

            ta = pst.tile([128, Bb * 2 * G2], F16, name="ta", tag="ta")
            tcv = pst.tile([128, Bb * G2], F16, name="tcv", tag="tcq")
            tc3 = tcv[:].rearrange("p (b g) -> p b g", b=Bb)
            tab = ta[:].rearrange("p (b k g) -> p b k g", b=Bb, k=2)
            eng(ENG_E3).tensor_add(
                tab, xsq_v2[:, :, 0:4:2, :], xsq_v2[:, :, 1:4:2, :]
            )
            eng(ENG_E3).tensor_add(tc3, tab[:, :, 0, :], tab[:, :, 1, :])
            eng(ENG_E3).tensor_add(v963[:, :, G1:GVv], tc3, xsq_v2[:, :, 4, :])

            # ---- variance + rsqrt (trees summed pre-scaled squares = var) ----
            GV = G1 + G2  # 96 v-segments per row-block
            inv96 = pst.tile([128, Bb * GV], F16, name="inv96", tag="inv96")
            _rsqrt(nc, inv96[:], var96[:], eps_t[:])
            i963 = inv96[:].rearrange("p (b g) -> p b g", b=Bb)
            # scal: f32 single-pass variance, inv_s f32 (ACT scale must be f32)
            m2 = pst.tile([128, Bb], F32, name="m2", tag="m2")
            nc.vector.tensor_mul(m2[:], nm_s[:], nm_s[:])
            var_s = pst.tile([128, Bb], F32, name="var_s", tag="var_s")
            nc.vector.tensor_sub(var_s[:], SSs[:], m2[:])
            inv_s = pst.tile([128, Bb], F32, name="inv_s", tag="inv_s")
            _rsqrt(nc, inv_s[:], var_s[:], eps_t[:])
            c_s = pst.tile([128, Bb], F32, name="c_s", tag="c_s")
            nc.vector.tensor_mul(c_s[:], nm_s[:], inv_s[:])

            # ---- normalize ----
            ot = po.tile([128, Bb * DIM], F16, name="ot", tag="o")
            o3 = ot[:].rearrange("p (b f) -> p b f", b=Bb)
            o_v1 = o3[:, :, S : S + V1W].rearrange("p b (k g) -> p b k g", k=D1)
            o_v2 = o3[:, :, S + V1W : DIM].rearrange("p b (k g) -> p b k g", k=D2)
            iv1h = i963[:, :, 0:G1].rearrange("p b (h g) -> p b h g", h=2)
            iv2 = i963[:, :, G1:GV]
            o_v1h = o_v1.rearrange("p b k (h g) -> p b k h g", h=2)
            xc_v1h = xc_v1.rearrange("p b k (h g) -> p b k h g", h=2)
            for k in range(D1):
                nc.vector.tensor_mul(
                    o_v1h[:, :, k, :, :], xc_v1h[:, :, k, :, :], iv1h
                )
            for k in range(D2):
                nc.vector.tensor_mul(o_v2[:, :, k, :], xc_v2[:, :, k, :], iv2)

            # scal: fused per-row t = Identity(x*inv + (-m*inv)) on ScalarE
            ts = pst.tile([128, Bb * S], F16, name="ts", tag="ts")
            for b in range(Bb):
                nc.scalar.activation(
                    ts[:, b * S : (b + 1) * S],
                    xt[:, b * DIM : b * DIM + S],
                    AF.Identity,
                    bias=c_s[:, b : b + 1],
                    scale=inv_s[:, b : b + 1],
                )
            u = pst.tile([128, Bb * S], F16, name="u", tag="u")
            u3 = u[:].rearrange("p (b f) -> p b f", b=Bb)
            nc.vector.tensor_mul(u3, ts[:].rearrange("p (b f) -> p b f", b=Bb), wb_b)
            nc.vector.tensor_add(o3[:, :, 0:S], u3, bb_b)

            nc.sync.dma_start(ov[i], ot[:])

    nc.compile()
    return nc


def _in_maps(x, weight, bias, rows):
    wb = np.ascontiguousarray(
        np.broadcast_to(np.tile(weight.astype(np.float16), B), (128, B * S))
    )
    bb = np.ascontiguousarray(
        np.broadcast_to(np.tile(bias.astype(np.float16), B), (128, B * S))
    )
    xp = np.ascontiguousarray(x[:, PERM], np.float32)
    return [
        {
            "x": xp[c * rows : (c + 1) * rows],
            "wb": wb,
            "bb": bb,
            "epsv": np.full((128, 1), EPS, np.float32),
        }
        for c in range(N_CORES)
    ]


_NC_CACHE = {}


def kernel(x, weight, bias):
    x = np.asarray(x, np.float32)
    weight = np.asarray(weight, np.float32)
    bias = np.asarray(bias, np.float32)
    rows = x.shape[0] // N_CORES
    key = (rows, B)
    if key not in _NC_CACHE:
        _NC_CACHE[key] = build_nc(rows=rows, b_blocks=B)
    nc = _NC_CACHE[key]
    res = run_bass_kernel_spmd(nc, _in_maps(x, weight, bias, rows), list(range(N_CORES)))
    out_p = np.concatenate(
        [res.results[c]["out"] for c in range(N_CORES)], axis=0
    ).astype(np.float32)
    out = np.empty_like(out_p)
    out[:, PERM] = out_p
    return out


# revision 14
# speedup vs baseline: 1.0522x; 1.0522x over previous
"""Equivariant LayerNorm (128x0e + 64x1o + 32x2e) Trainium2 Bass kernel.

Sharding: pure data parallel over 8 NeuronCores, 32768 rows each.

Key design (v2 — engineered against measured per-engine rates):
  * Host permutes the v1/v2 feature blocks to k-major ("interleaved") order so
    every per-segment broadcast on the device has 64/32-wide consecutive inner
    runs -> fp16 tensor_tensor ops hit the DVE 2x packed mode.
  * Output is stored as fp16 (rel err ~8e-4, gate 2e-2) halving store traffic.
  * v1/v2 blocks: two-pass variance. Pass 1 sums in f32 (mean must be exact:
    tiny-variance segments amplify mean error by 1/sqrt(var+eps)).  The
    centered tensor xc is cached as fp16 (no cancellation once centered), so
    the square, the sum-of-squares trees and the normalize multiply all run
    at 2x on fp16.
  * scal block (d=128): single-pass f32 variance (E[x^2]-m^2 is safe at d=128
    since row variance is never tiny) + per-row-fused ACT normalize
    t = Identity(x*inv + (-m*inv)) with per-partition scale/bias.
  * GPSIMD takes the two big mixed-dtype centering adds (f32+f32->fp16, 1x on
    DVE anyway); ScalarE takes squares + rsqrt + the fused scal normalize.
"""

import sys

import numpy as np

try:
    import concourse  # noqa: F401
except ImportError:  # pragma: no cover
    sys.path.insert(0, "/opt/trn_rl_repo")

from contextlib import ExitStack

import concourse.bacc as bacc
import concourse.bass as bass
import concourse.mybir as mybir
import concourse.tile as tile
from concourse.bass_utils import run_bass_kernel_spmd

F32 = mybir.dt.float32
F16 = mybir.dt.float16
AF = mybir.ActivationFunctionType
AXX = mybir.AxisListType.X
ALU = mybir.AluOpType

N = 262144
DIM = 480
S = 128
G1, D1 = 64, 3
G2, D2 = 32, 5
GS = 1 + G1 + G2  # 97 segments per row (seg 0 = the 128 scalar cols)
EPS = 1e-5

N_CORES = 8
PRI_LOAD = 100   # schedule loads ~2 tiles early
PRI_STAGE = 50   # sums/center/squares ~1 tile early
ROWS = N // N_CORES  # 32768
B = 8  # row-blocks per SBUF tile
TILE_ROWS = 128 * B

# engine assignment knobs (tuned against HW)
ENG_C1 = "vector"  # center v1
ENG_C2 = "vector"  # center v2
ENG_E2 = "vector"  # SS_v1 tree
ENG_E3 = "vector"  # SS_v2 tree
ENG_XSQS = "scalar"  # scal x^2 (f32)


def _perm():
    """Device column permutation: v-blocks to k-major (interleaved) order.
    perm[dev_col] = orig_col."""
    p = np.arange(DIM)
    for k in range(D1):
        for g in range(G1):
            p[S + k * G1 + g] = S + g * D1 + k
    off = S + G1 * D1
    for k in range(D2):
        for g in range(G2):
            p[off + k * G2 + g] = off + g * D2 + k
    return p


PERM = _perm()


def _rsqrt(nc, out_ap, in_ap, bias_ap):
    """out = 1/sqrt(in + bias) on ScalarE. The bass wrapper rejects Rsqrt on
    accuracy grounds; measured on this HW it is ~4e-5 max rel err, far below
    the tolerance here."""
    eng = nc.scalar
    return eng.add_instruction(
        mybir.InstActivation(
            name=nc.get_next_instruction_name(),
            func=AF.Rsqrt,
            ins=[
                eng.lower_ap(in_ap),
                eng.lower_ap(bias_ap),
                mybir.ImmediateValue(dtype=F32, value=1.0),
                mybir.ImmediateValue(dtype=F32, value=0.0),
            ],
            outs=[eng.lower_ap(out_ap)],
        )
    )


def build_nc(rows=ROWS, b_blocks=B):
    nc = bacc.Bacc("TRN2", target_bir_lowering=False, debug=False)
    Bb = b_blocks
    trows = 128 * Bb
    assert rows % trows == 0
    ntiles = rows // trows
    V1W = G1 * D1  # 192
    V2W = G2 * D2  # 160
    VW = V1W + V2W  # 352

    x_d = nc.dram_tensor("x", [rows, DIM], F32, kind="ExternalInput").ap()
    wb_d = nc.dram_tensor("wb", [128, b_blocks * S], F16, kind="ExternalInput").ap()
    bb_d = nc.dram_tensor("bb", [128, b_blocks * S], F16, kind="ExternalInput").ap()
    eps_d = nc.dram_tensor("epsv", [128, 1], F32, kind="ExternalInput").ap()
    out_d = nc.dram_tensor("out", [rows, DIM], F16, kind="ExternalOutput").ap()

    # p-major row blocking: row = n*(128*B) + p*B + b -> each partition's tile
    # slice is one contiguous run in DRAM (fat DMA descriptors)
    xv = x_d.rearrange("(n p b) f -> n p (b f)", p=128, b=Bb)
    ov = out_d.rearrange("(n p b) f -> n p (b f)", p=128, b=Bb)

    def eng(name):
        return getattr(nc, name)

    with tile.TileContext(nc) as tc, ExitStack() as ctx:
        const = ctx.enter_context(tc.tile_pool(name="const", bufs=1))
        px = ctx.enter_context(tc.tile_pool(name="px", bufs=4))
        pxc = ctx.enter_context(tc.tile_pool(name="pxc", bufs=4))
        pxsq = ctx.enter_context(tc.tile_pool(name="pxsq", bufs=3))
        po = ctx.enter_context(tc.tile_pool(name="po", bufs=3))
        pst = ctx.enter_context(tc.tile_pool(name="pst", bufs=3))

        wb_t = const.tile([128, Bb * S], F16, name="wbt", tag="wb")
        nc.sync.dma_start(wb_t[:], wb_d)
        bb_t = const.tile([128, Bb * S], F16, name="bbt", tag="bb")
        nc.sync.dma_start(bb_t[:], bb_d)
        eps_t = const.tile([128, 1], F32, name="epst", tag="epsv")
        nc.sync.dma_start(eps_t[:], eps_d)

        wb_b = wb_t[:].rearrange("p (b f) -> p b f", b=Bb)
        bb_b = bb_t[:].rearrange("p (b f) -> p b f", b=Bb)

        for i in range(ntiles):
            xt = px.tile([128, Bb * DIM], F32, name="xt", tag="x")
            with tc.high_priority(offset=PRI_LOAD):
                nc.sync.dma_start(xt[:], xv[i])
            x3 = xt[:].rearrange("p (b f) -> p b f", b=Bb)
            x_s = x3[:, :, 0:S]
            x_v1 = x3[:, :, S : S + V1W].rearrange("p b (k g) -> p b k g", k=D1)
            x_v2 = x3[:, :, S + V1W : DIM].rearrange("p b (k g) -> p b k g", k=D2)

            # ---- pass 1: segment sums (f32) ----
            hp = tc.high_priority(offset=PRI_STAGE)
            hp.__enter__()
            nSs = pst.tile([128, Bb], F32, name="nSs", tag="nSs")  # -sum(scal)
            nc.vector.tensor_reduce(
                nSs[:], x_s, axis=AXX, op=ALU.add, negate=True
            )
            nm_s = pst.tile([128, Bb], F32, name="nm_s", tag="nm_s")
            nc.vector.tensor_scalar_mul(nm_s[:], nSs[:], 1.0 / S)

            tr1 = pst.tile([128, Bb * G1], F32, name="tr1", tag="tr1")
            t14 = tr1[:].rearrange("p (b h g) -> p b h g", b=Bb, h=2)
            xv14 = x_v1.rearrange("p b k (h g) -> p b k h g", h=2)
            nc.vector.tensor_add(t14, xv14[:, :, 0, :, :], xv14[:, :, 1, :, :])
            Sv1 = pst.tile([128, Bb * G1], F32, name="Sv1", tag="Sv1")
            S14 = Sv1[:].rearrange("p (b h g) -> p b h g", b=Bb, h=2)
            nc.vector.tensor_add(S14, t14, xv14[:, :, 2, :, :])
            nm_v1 = pst.tile([128, Bb * G1], F32, name="nm_v1", tag="nm_v1")
            nc.vector.tensor_scalar_mul(nm_v1[:], Sv1[:], -1.0 / D1)
            nm1_b = (
                nm_v1[:]
                .rearrange("p (b o g) -> p b o g", b=Bb, o=1)
                .broadcast_to([128, Bb, D1, G1])
            )

            ra = pst.tile([128, Bb * 2 * G2], F32, name="ra", tag="ra")
            rc = pst.tile([128, Bb * G2], F32, name="rc", tag="rc")
            rc3 = rc[:].rearrange("p (b g) -> p b g", b=Bb)
            Sv2 = pst.tile([128, Bb * G2], F32, name="Sv2", tag="Sv2")
            S23 = Sv2[:].rearrange("p (b g) -> p b g", b=Bb)
            rab = ra[:].rearrange("p (b k g) -> p b k g", b=Bb, k=2)
            nc.vector.tensor_add(
                rab, x_v2[:, :, 0:4:2, :], x_v2[:, :, 1:4:2, :]
            )
            nc.vector.tensor_add(rc3, rab[:, :, 0, :], rab[:, :, 1, :])
            nc.vector.tensor_add(S23, rc3, x_v2[:, :, 4, :])
            nm_v2 = pst.tile([128, Bb * G2], F32, name="nm_v2", tag="nm_v2")
            nc.vector.tensor_scalar_mul(nm_v2[:], Sv2[:], -1.0 / D2)
            nm2_b = (
                nm_v2[:]
                .rearrange("p (b o g) -> p b o g", b=Bb, o=1)
                .broadcast_to([128, Bb, D2, G2])
            )

            # ---- center v-blocks -> fp16 xc (compact [p, b, 352]) ----
            xc = pxc.tile([128, Bb * VW], F16, name="xc", tag="xc")
            c3 = xc[:].rearrange("p (b f) -> p b f", b=Bb)
            xc_v1 = c3[:, :, 0:V1W].rearrange("p b (k g) -> p b k g", k=D1)
            xc_v2 = c3[:, :, V1W:VW].rearrange("p b (k g) -> p b k g", k=D2)
            eng(ENG_C1).tensor_add(xc_v1, x_v1, nm1_b)
            eng(ENG_C2).tensor_add(xc_v2, x_v2, nm2_b)

            # ---- squares ----
            xsqs = pxsq.tile([128, Bb * S], F32, name="xsqs", tag="xsqs")
            sq3 = xsqs[:].rearrange("p (b f) -> p b f", b=Bb)
            # scale 1/sqrt(S): the reduce then yields SS/S = E[x^2] directly
            nc.scalar.activation(sq3, x_s, AF.Square, scale=float(1.0 / np.sqrt(S)))
            xsq = pxsq.tile([128, Bb * VW], F16, name="xsq", tag="xsq")
            q3 = xsq[:].rearrange("p (b f) -> p b f", b=Bb)
            nc.scalar.activation(
                q3[:, :, 0:V1W], c3[:, :, 0:V1W], AF.Square,
                scale=float(1.0 / np.sqrt(D1)),
            )
            nc.scalar.activation(
                q3[:, :, V1W:VW], c3[:, :, V1W:VW], AF.Square,
                scale=float(1.0 / np.sqrt(D2)),
            )
            xsq_v1 = q3[:, :, 0:V1W].rearrange("p b (k g) -> p b k g", k=D1)
            xsq_v2 = q3[:, :, V1W:VW].rearrange("p b (k g) -> p b k g", k=D2)
            hp.__exit__(None, None, None)

            # ---- pass 2: sums of squares ----
            SSs = pst.tile([128, Bb], F32, name="SSs", tag="SSs")
            nc.vector.tensor_reduce(SSs[:], sq3, axis=AXX, op=ALU.add)

            GVv = G1 + G2
            var96 = pst.tile([128, Bb * GVv], F16, name="var96", tag="var96")
            v963 = var96[:].rearrange("p (b g) -> p b g", b=Bb)
            tq1 = pst.tile([128, Bb * G1], F16, name="tq1", tag="tq1")
            tq14 = tq1[:].rearrange("p (b h g) -> p b h g", b=Bb, h=2)
            xq14 = xsq_v1.rearrange("p b k (h g) -> p b k h g", h=2)
            eng(ENG_E2).tensor_add(tq14, xq14[:, :, 0, :, :], xq14[:, :, 1, :, :])
            v96h = v963[:, :, 0:G1].rearrange("p b (h g) -> p b h g", h=2)
            eng(ENG_E2).tensor_add(v96h, tq14, xq14[:, :, 2, :, :])

            ta = pst.tile([128, Bb * 2 * G2], F16, name="ta", tag="ta")
            tcv = pst.tile([128, Bb * G2], F16, name="tcv", tag="tcq")
            tc3 = tcv[:].rearrange("p (b g) -> p b g", b=Bb)
            tab = ta[:].rearrange("p (b k g) -> p b k g", b=Bb, k=2)
            eng(ENG_E3).tensor_add(
                tab, xsq_v2[:, :, 0:4:2, :], xsq_v2[:, :, 1:4:2, :]
            )
            eng(ENG_E3).tensor_add(tc3, tab[:, :, 0, :], tab[:, :, 1, :])
            eng(ENG_E3).tensor_add(v963[:, :, G1:GVv], tc3, xsq_v2[:, :, 4, :])

            # ---- variance + rsqrt (trees summed pre-scaled squares = var) ----
            GV = G1 + G2  # 96 v-segments per row-block
            inv96 = pst.tile([128, Bb * GV], F16, name="inv96", tag="inv96")
            _rsqrt(nc, inv96[:], var96[:], eps_t[:])
            i963 = inv96[:].rearrange("p (b g) -> p b g", b=Bb)
            # scal: f32 single-pass variance, inv_s f32 (ACT scale must be f32)
            m2 = pst.tile([128, Bb], F32, name="m2", tag="m2")
            nc.vector.tensor_mul(m2[:], nm_s[:], nm_s[:])
            var_s = pst.tile([128, Bb], F32, name="var_s", tag="var_s")
            nc.vector.tensor_sub(var_s[:], SSs[:], m2[:])
            inv_s = pst.tile([128, Bb], F32, name="inv_s", tag="inv_s")
            _rsqrt(nc, inv_s[:], var_s[:], eps_t[:])
            c_s = pst.tile([128, Bb], F32, name="c_s", tag="c_s")
            nc.vector.tensor_mul(c_s[:], nm_s[:], inv_s[:])

            # ---- normalize ----
            ot = po.tile([128, Bb * DIM], F16, name="ot", tag="o")
            o3 = ot[:].rearrange("p (b f) -> p b f", b=Bb)
            o_v1 = o3[:, :, S : S + V1W].rearrange("p b (k g) -> p b k g", k=D1)
            o_v2 = o3[:, :, S + V1W : DIM].rearrange("p b (k g) -> p b k g", k=D2)
            iv1h = i963[:, :, 0:G1].rearrange("p b (h g) -> p b h g", h=2)
            iv2 = i963[:, :, G1:GV]
            o_v1h = o_v1.rearrange("p b k (h g) -> p b k h g", h=2)
            xc_v1h = xc_v1.rearrange("p b k (h g) -> p b k h g", h=2)
            for k in range(D1):
                nc.vector.tensor_mul(
                    o_v1h[:, :, k, :, :], xc_v1h[:, :, k, :, :], iv1h
                )
            for k in range(D2):
                nc.vector.tensor_mul(o_v2[:, :, k, :], xc_v2[:, :, k, :], iv2)

            # scal: fused per-row t = Identity(x*inv + (-m*inv)) on ScalarE
            ts = pst.tile([128, Bb * S], F16, name="ts", tag="ts")
            for b in range(Bb):
                nc.scalar.activation(
                    ts[:, b * S : (b + 1) * S],
                    xt[:, b * DIM : b * DIM + S],
                    AF.Identity,
                    bias=c_s[:, b : b + 1],
                    scale=inv_s[:, b : b + 1],
                )
            u = pst.tile([128, Bb * S], F16, name="u", tag="u")
            u3 = u[:].rearrange("p (b f) -> p b f", b=Bb)
            nc.vector.tensor_mul(u3, ts[:].rearrange("p (b f) -> p b f", b=Bb), wb_b)
            nc.vector.tensor_add(o3[:, :, 0:S], u3, bb_b)

            nc.sync.dma_start(ov[i], ot[:])

    nc.compile()
    return nc


def _in_maps(x, weight, bias, rows):
    wb = np.ascontiguousarray(
        np.broadcast_to(np.tile(weight.astype(np.float16), B), (128, B * S))
    )
    bb = np.ascontiguousarray(
        np.broadcast_to(np.tile(bias.astype(np.float16), B), (128, B * S))
    )
    xp = np.ascontiguousarray(x[:, PERM], np.float32)
    return [
        {
            "x": xp[c * rows : (c + 1) * rows],
            "wb": wb,
            "bb": bb,
            "epsv": np.full((128, 1), EPS, np.float32),
        }
        for c in range(N_CORES)
    ]


_NC_CACHE = {}


def kernel(x, weight, bias):
    x = np.asarray(x, np.float32)
    weight = np.asarray(weight, np.float32)
    bias = np.asarray(bias, np.float32)
    rows = x.shape[0] // N_CORES
    key = (rows, B)
    if key not in _NC_CACHE:
        _NC_CACHE[key] = build_nc(rows=rows, b_blocks=B)
    nc = _NC_CACHE[key]
    res = run_bass_kernel_spmd(nc, _in_maps(x, weight, bias, rows), list(range(N_CORES)))
    out_p = np.concatenate(
        [res.results[c]["out"] for c in range(N_CORES)], axis=0
    ).astype(np.float32)
    out = np.empty_like(out_p)
    out[:, PERM] = out_p
    return out
